# revision 1
# baseline (speedup 1.0000x reference)
"""BitNet attention (GQA + RoPE) on 8 Trainium2 NeuronCores.

Tensor-parallel over heads: core c owns q-heads [4c, 4c+4), kv-head c.
Each core computes q/k/v projections (ternary BitNet weights), RoPE,
attention for its heads, and a row-parallel partial of the Wo
projection; the host sums the 8 partials.

All matmuls run as float32r (fp32 data on the PE fast path, ~1e-4
matmul error at bf16-like speed for moving dims >= 256).

Layout notes (per core):
  qT   [128, 2, 2048]  head-pair p: head 2p on partitions 0:64, head
                       2p+1 on 64:128; RoPE already applied.
  kTd  [128, 2048]     the kv head, duplicated on both partition
                       halves so the K=64 score matmuls can run as a
                       row-tiled concurrent pair (tile rows 0 and 64).
  V    [128, 16, 65]   [sk-chunk, 65]; col 64 is ones so the AV matmul
                       also produces the softmax denominators.
  aoT  [128, 2, 2048]  normalized attention outputs, o-major, the lhsT
                       of the Wo matmul.
"""

import sys

if "/opt/trn_rl_repo" not in sys.path:
    sys.path.insert(0, "/opt/trn_rl_repo")

import numpy as np

import concourse.bass as bass
from concourse import bacc, mybir
from concourse.bass import ts
from concourse.bass_utils import run_bass_kernel_spmd
from concourse.masks import make_identity
from concourse.tile import TileContext

F32 = mybir.dt.float32
F32R = mybir.dt.float32r

S = 2048
H = 2048
N_HEADS = 32
N_KV = 8
D = 64
NCORES = 8
HPC = N_HEADS // NCORES  # 4 q heads per core
OC = HPC * D  # 256 output dims per core
NB = S // 256  # 8 s-blocks of 256
HC = H // 128  # 16 hidden chunks

LAST_EXEC_NS = None
_CACHE = {}


def _ternarize(w):
    w = np.asarray(w, np.float32)
    s = (np.abs(w).mean() + np.float32(1e-6)).astype(np.float32)
    t = np.round(np.clip(w / s, np.float32(-1.0), np.float32(1.0))).astype(np.float32)
    return t, float(s)


def _build_program(s_qk, s_vo):
    nc = bacc.Bacc("TRN2", target_bir_lowering=False, debug=False, num_devices=NCORES)

    xt = nc.dram_tensor("xt", [NB, 128, HC, 256], F32R, kind="ExternalInput")
    wq = nc.dram_tensor("wq_t", [128, HC, OC], F32R, kind="ExternalInput")
    wkv = nc.dram_tensor("wkv_t", [128, HC, 128], F32R, kind="ExternalInput")
    wo = nc.dram_tensor("wo_t", [128, 2, H], F32R, kind="ExternalInput")
    cos_d = nc.dram_tensor("cos_t", [128, S], F32, kind="ExternalInput")
    sin_d = nc.dram_tensor("sin_t", [128, S], F32, kind="ExternalInput")
    mask_d = nc.dram_tensor("mask_t", [128, HC], F32, kind="ExternalInput")
    ones_d = nc.dram_tensor("ones_t", [128, HC], F32R, kind="ExternalInput")
    outp = nc.dram_tensor("outp", [S, H], F32, kind="ExternalOutput")

    EXP = mybir.ActivationFunctionType.Exp
    MUL = mybir.AluOpType.mult
    ADD = mybir.AluOpType.add

    with TileContext(nc) as tc:
        with tc.tile_pool(name="persist", bufs=1) as persist:
            qT = persist.tile([128, 2, S], F32R)
            kTd = persist.tile([128, S], F32R)
            V = persist.tile([128, HC, 65], F32R)
            aoT = persist.tile([128, 2, S], F32R)
            ident = persist.tile([128, 128], F32)
            make_identity(nc, ident[:])
            mask_sb = persist.tile([128, HC], F32)
            nc.sync.dma_start(mask_sb[:], mask_d[:])
            cos_sb = persist.tile([128, S], F32)
            nc.sync.dma_start(cos_sb[:], cos_d[:])
            sin_sb = persist.tile([128, S], F32)
            nc.sync.dma_start(sin_sb[:], sin_d[:])
            for i in range(HC):
                nc.sync.dma_start(V[:, i, 64:65], ones_d[:, i : i + 1])

            # ---- Phase 1: projections + RoPE ----
            with (
                tc.tile_pool(name="ph1w", bufs=1) as ph1w,
                tc.tile_pool(name="xtp", bufs=3) as xtp,
                tc.tile_pool(name="ph1t", bufs=3) as ph1t,
            ):
                wq_sb = ph1w.tile([128, HC, OC], F32R)
                nc.sync.dma_start(wq_sb[:], wq[:])
                wkv_sb = ph1w.tile([128, HC, 128], F32R)
                nc.sync.dma_start(wkv_sb[:], wkv[:])
                vT = ph1w.tile([64, S], F32)

                with tc.tile_pool(name="ps1", bufs=2, space="PSUM") as ps1:
                    for b in range(NB):
                        xt_t = xtp.tile([128, HC, 256], F32R, tag="xt")
                        nc.sync.dma_start(xt_t[:], xt[b])
                        pq0 = ps1.tile([128, 256], F32, tag="q0")
                        pq1 = ps1.tile([128, 256], F32, tag="q1")
                        pkv = ps1.tile([128, 256], F32, tag="kv")
                        for c in range(HC):
                            st, sp = c == 0, c == HC - 1
                            nc.tensor.matmul(
                                pq0[:], wq_sb[:, c, 0:128], xt_t[:, c, :], start=st, stop=sp
                            )
                            nc.tensor.matmul(
                                pq1[:], wq_sb[:, c, 128:256], xt_t[:, c, :], start=st, stop=sp
                            )
                            nc.tensor.matmul(
                                pkv[:], wkv_sb[:, c, :], xt_t[:, c, :], start=st, stop=sp
                            )
                        sb = ts(b, 256)
                        for p, pq in ((0, pq0), (1, pq1)):
                            rot = ph1t.tile([128, 256], F32, tag="rot")
                            nc.vector.tensor_copy(rot[0:32, :], pq[32:64, :])
                            nc.vector.tensor_copy(rot[32:64, :], pq[0:32, :])
                            nc.vector.tensor_copy(rot[64:96, :], pq[96:128, :])
                            nc.vector.tensor_copy(rot[96:128, :], pq[64:96, :])
                            qc = ph1t.tile([128, 256], F32, tag="qc")
                            nc.vector.tensor_tensor(qc[:], pq[:], cos_sb[:, sb], MUL)
                            qs = ph1t.tile([128, 256], F32, tag="qs")
                            nc.vector.tensor_tensor(qs[:], rot[:], sin_sb[:, sb], MUL)
                            nc.vector.tensor_tensor(qT[:, p, sb], qc[:], qs[:], ADD)
                        rotk = ph1t.tile([64, 256], F32, tag="rotk")
                        nc.vector.tensor_copy(rotk[0:32, :], pkv[32:64, :])
                        nc.vector.tensor_copy(rotk[32:64, :], pkv[0:32, :])
                        kc = ph1t.tile([64, 256], F32, tag="kc")
                        nc.vector.tensor_tensor(kc[:], pkv[0:64, :], cos_sb[0:64, sb], MUL)
                        ks = ph1t.tile([64, 256], F32, tag="ks")
                        nc.vector.tensor_tensor(ks[:], rotk[:], sin_sb[0:64, sb], MUL)
                        nc.vector.tensor_tensor(kTd[0:64, sb], kc[:], ks[:], ADD)
                        nc.vector.tensor_tensor(kTd[64:128, sb], kc[:], ks[:], ADD)
                        nc.vector.tensor_copy(vT[:, sb], pkv[64:128, :])

                with tc.tile_pool(name="psvt", bufs=2, space="PSUM") as psvt:
                    for i in range(HC):
                        pt = psvt.tile([128, 64], F32, tag="vt")
                        nc.tensor.transpose(pt[:], vT[:, ts(i, 128)], ident[0:64, 0:64])
                        nc.vector.tensor_scalar_mul(V[:, i, 0:64], pt[:], s_vo)

            # ---- Phase 2: attention ----
            with (
                tc.tile_pool(name="expp", bufs=2) as expp,
                tc.tile_pool(name="ph2t", bufs=3) as ph2t,
                tc.tile_pool(name="csd", bufs=4, space="DRAM") as csd,
                tc.tile_pool(name="pssc", bufs=2, space="PSUM") as pssc,
                tc.tile_pool(name="psav", bufs=2, space="PSUM") as psav,
            ):
                for p in range(2):
                    for j in range(NB):
                        jb = ts(j, 256)
                        eA = expp.tile([128, HC, 256], F32R, tag="eA")
                        eB = expp.tile([128, HC, 256], F32R, tag="eB")
                        for i in range(HC):
                            psA = pssc.tile([128, 256], F32, tag="sA")
                            psB = pssc.tile([128, 256], F32, tag="sB")
                            nc.tensor.matmul(
                                psA[:], kTd[0:64, ts(i, 128)], qT[0:64, p, jb],
                                start=True, stop=True,
                            )
                            nc.tensor.matmul(
                                psB[:], kTd[64:128, ts(i, 128)], qT[64:128, p, jb],
                                start=True, stop=True,
                            )
                            nc.scalar.activation(
                                eA[:, i, :], psA[:], EXP,
                                bias=mask_sb[:, i : i + 1], scale=s_qk,
                            )
                            nc.scalar.activation(
                                eB[:, i, :], psB[:], EXP,
                                bias=mask_sb[:, i : i + 1], scale=s_qk,
                            )
                        pA = psav.tile([65, 256], F32, tag="avA")
                        pB = psav.tile([65, 256], F32, tag="avB")
                        for i in range(HC):
                            st, sp = i == 0, i == HC - 1
                            nc.tensor.matmul(pA[:], V[:, i, :], eA[:, i, :], start=st, stop=sp)
                            nc.tensor.matmul(pB[:], V[:, i, :], eB[:, i, :], start=st, stop=sp)
                        for h, pav in ((0, pA), (1, pB)):
                            cs = ph2t.tile([1, 256], F32, tag="cs")
                            nc.vector.tensor_copy(cs[:], pav[64:65, :])
                            cs_dram = csd.tile([1, 256], F32, tag="csd")
                            nc.sync.dma_start(cs_dram[:], cs[:])
                            cb = ph2t.tile([64, 256], F32, tag="cb")
                            nc.sync.dma_start(cb[:], cs_dram[:].to_broadcast((64, 256)))
                            rc = ph2t.tile([64, 256], F32, tag="rc")
                            nc.vector.reciprocal(rc[:], cb[:])
                            nc.vector.tensor_tensor(
                                aoT[h * 64 : (h + 1) * 64, p, jb], pav[0:64, :], rc[:], MUL
                            )

            # ---- Phase 3: output projection (row-parallel partial) ----
            with (
                tc.tile_pool(name="ph3w", bufs=1) as ph3w,
                tc.tile_pool(name="osp", bufs=4) as osp,
                tc.tile_pool(name="pso", bufs=3, space="PSUM") as pso_,
            ):
                wo_sb = ph3w.tile([128, 2, H], F32R)
                nc.sync.dma_start(wo_sb[:], wo[:])
                for jq in range(16):
                    for hb in range(4):
                        po = pso_.tile([128, 512], F32, tag="po")
                        nc.tensor.matmul(
                            po[:], aoT[:, 0, ts(jq, 128)], wo_sb[:, 0, ts(hb, 512)],
                            start=True, stop=False,
                        )
                        nc.tensor.matmul(
                            po[:], aoT[:, 1, ts(jq, 128)], wo_sb[:, 1, ts(hb, 512)],
                            start=False, stop=True,
                        )
                        ob = osp.tile([128, 512], F32, tag="ob")
                        if (jq * 4 + hb) % 2 == 0:
                            nc.vector.tensor_copy(ob[:], po[:])
                        else:
                            nc.scalar.copy(ob[:], po[:])
                        nc.sync.dma_start(outp[ts(jq, 128), ts(hb, 512)], ob[:])

    nc.compile()
    return nc


def kernel(
    hidden_states,
    attention_mask,
    position_ids,
    wq,
    wk,
    wv,
    wo,
    _trace=False,
):
    global LAST_EXEC_NS
    x = np.asarray(hidden_states, np.float32)[0]  # [S, H]
    mask = np.asarray(attention_mask, np.float32)[0]  # [S]
    pos = np.asarray(position_ids)[0].astype(np.float32)  # [S]

    wq_t, s_q = _ternarize(wq)
    wk_t, s_k = _ternarize(wk)
    wv_t, s_v = _ternarize(wv)
    wo_t, s_o = _ternarize(wo)
    s_qk = float(np.float32(s_q) * np.float32(s_k) / np.float32(8.0))
    s_vo = float(np.float32(s_v) * np.float32(s_o))

    key = ("v1", s_qk, s_vo)
    if key not in _CACHE:
        _CACHE.clear()
        _CACHE[key] = _build_program(s_qk, s_vo)
    nc = _CACHE[key]

    # shared inputs
    xt_host = np.ascontiguousarray(
        x.T.reshape(HC, 128, NB, 256).transpose(2, 1, 0, 3)
    )
    inv = (1.0 / (10000.0 ** (np.arange(0, D, 2, dtype=np.float32) / np.float32(D)))).astype(np.float32)
    fr = pos[:, None] * inv[None, :]  # [S, 32]
    emb = np.concatenate([fr, fr], axis=1)  # [S, 64]
    cos64 = np.cos(emb).astype(np.float32)
    sin64 = np.sin(emb).astype(np.float32)
    sin64[:, : D // 2] *= -1.0
    cos128 = np.ascontiguousarray(np.vstack([cos64.T, cos64.T]))  # [128, S]
    sin128 = np.ascontiguousarray(np.vstack([sin64.T, sin64.T]))
    mask_r = np.ascontiguousarray(mask.reshape(HC, 128).T)  # [128, HC]
    ones_r = np.ones((128, HC), np.float32)

    in_maps = []
    for c in range(NCORES):
        wq_c = np.ascontiguousarray(
            wq_t[c * OC : (c + 1) * OC, :].T.reshape(HC, 128, OC).transpose(1, 0, 2)
        )
        wk_c = wk_t[c * D : (c + 1) * D, :].T  # [H, 64]
        wv_c = wv_t[c * D : (c + 1) * D, :].T
        wkv_c = np.ascontiguousarray(
            np.concatenate([wk_c, wv_c], axis=1).reshape(HC, 128, 128).transpose(1, 0, 2)
        )
        wo_c = np.ascontiguousarray(
            wo_t[:, c * OC : (c + 1) * OC].T.reshape(2, 128, H).transpose(1, 0, 2)
        )
        in_maps.append(
            {
                "xt": xt_host,
                "wq_t": wq_c,
                "wkv_t": wkv_c,
                "wo_t": wo_c,
                "cos_t": cos128,
                "sin_t": sin128,
                "mask_t": mask_r,
                "ones_t": ones_r,
            }
        )

    res = run_bass_kernel_spmd(
        nc, in_maps, core_ids=list(range(NCORES)), trace=bool(_trace)
    )
    LAST_EXEC_NS = res.exec_time_ns

    out = res.results[0]["outp"].astype(np.float32)
    for c in range(1, NCORES):
        out = out + res.results[c]["outp"]
    return out.reshape(1, S, H).astype(np.float32)


# revision 2
# speedup vs baseline: 1.2975x; 1.2975x over previous
"""BitNet attention (GQA + RoPE) on 8 Trainium2 NeuronCores.

Tensor-parallel over heads: core c owns q-heads [4c, 4c+4), kv-head c.
Each core computes q/k/v projections (ternary BitNet weights), RoPE,
attention for its heads, and a row-parallel partial of the Wo
projection; the host sums the 8 partials.

Dtype strategy (measured on HW):
  - projections / AV / Wo matmuls: float32r (~fp32 precision at
    1 cyc/row for moving dims >= 256)
  - scores matmuls: bf16, K=64 row-tiled so the two heads of a pair
    run concurrently in the PE array (55 ns per [64x128x512] matmul)
  - fp32r with row tiling is ~20x slow on HW; bf16 scores cost ~9e-4
    relative error end-to-end.

The attention mask is folded into the V tile: attn = exp(s*qk + m) =
exp(m)*exp(s*qk), so V rows and the denominator-ones column are
pre-scaled by exp(mask) and the EXP activation needs no bias.

Layout notes (per core):
  qT   [128, 2, 2048]  head-pair p: head 2p on partitions 0:64, head
                       2p+1 on 64:128; RoPE applied; bf16.
  kTd  [128, 2048]     kv head duplicated on both partition halves
                       (lhsT of both row-tiled score matmuls); bf16.
  V    [128, 16, 65]   [sk-chunk, 65] f32r; col 64 = exp(mask) so the
                       AV matmul also emits softmax denominators;
                       cols 0:64 scaled by exp(mask)*s_v*s_o.
  aoT  [128, 2, 2048]  normalized attention outputs, o-major, f32r,
                       lhsT of the Wo matmul.
"""

import sys

if "/opt/trn_rl_repo" not in sys.path:
    sys.path.insert(0, "/opt/trn_rl_repo")

import numpy as np

import concourse.bass as bass
from concourse import bacc, mybir
from concourse.bass import ts
from concourse.bass_utils import run_bass_kernel_spmd
from concourse.masks import make_identity
from concourse.tile import TileContext

F32 = mybir.dt.float32
F32R = mybir.dt.float32r
BF16 = mybir.dt.bfloat16

S = 2048
H = 2048
N_HEADS = 32
N_KV = 8
D = 64
NCORES = 8
HPC = N_HEADS // NCORES  # 4 q heads per core
OC = HPC * D  # 256 output dims per core
NB = S // 512  # 4 s-blocks of 512
HC = H // 128  # 16 hidden chunks

LAST_EXEC_NS = None
LAST_TRACE = None
_CACHE = {}


def _ternarize(w):
    w = np.asarray(w, np.float32)
    s = (np.abs(w).mean() + np.float32(1e-6)).astype(np.float32)
    t = np.round(np.clip(w / s, np.float32(-1.0), np.float32(1.0))).astype(np.float32)
    return t, float(s)


def _build_program(s_qk):
    nc = bacc.Bacc("TRN2", target_bir_lowering=False, debug=False, num_devices=NCORES)

    xt = nc.dram_tensor("xt", [NB, 128, HC, 512], F32R, kind="ExternalInput")
    wq = nc.dram_tensor("wq_t", [128, HC, OC], F32R, kind="ExternalInput")
    wkv = nc.dram_tensor("wkv_t", [128, HC, 128], F32R, kind="ExternalInput")
    wo = nc.dram_tensor("wo_t", [128, 2, H], F32R, kind="ExternalInput")
    cos_d = nc.dram_tensor("cos_t", [128, S], F32, kind="ExternalInput")
    sin_d = nc.dram_tensor("sin_t", [128, S], F32, kind="ExternalInput")
    emv_d = nc.dram_tensor("emv_t", [128, HC], F32, kind="ExternalInput")
    em_d = nc.dram_tensor("em_t", [128, HC], F32R, kind="ExternalInput")
    outp = nc.dram_tensor("outp", [S, H], F32, kind="ExternalOutput")

    EXP = mybir.ActivationFunctionType.Exp
    MUL = mybir.AluOpType.mult
    ADD = mybir.AluOpType.add

    with TileContext(nc) as tc:
        with tc.tile_pool(name="persist", bufs=1) as persist:
            qT = persist.tile([128, 2, S], BF16)
            kTd = persist.tile([128, S], BF16)
            V = persist.tile([128, HC, 65], F32R)
            aoT = persist.tile([128, 2, S], F32R)
            for i in range(HC):
                nc.sync.dma_start(V[:, i, 64:65], em_d[:, i : i + 1])

            # ---- Phase 1: projections + RoPE ----
            with (
                tc.tile_pool(name="ph1w", bufs=1) as ph1w,
                tc.tile_pool(name="xtp", bufs=2) as xtp,
                tc.tile_pool(name="ph1t", bufs=3) as ph1t,
            ):
                wq_sb = ph1w.tile([128, HC, OC], F32R)
                nc.sync.dma_start(wq_sb[:], wq[:])
                wkv_sb = ph1w.tile([128, HC, 128], F32R)
                nc.sync.dma_start(wkv_sb[:], wkv[:])
                cos_sb = ph1w.tile([128, S], F32)
                nc.sync.dma_start(cos_sb[:], cos_d[:])
                sin_sb = ph1w.tile([128, S], F32)
                nc.sync.dma_start(sin_sb[:], sin_d[:])
                emv_sb = ph1w.tile([128, HC], F32)
                nc.sync.dma_start(emv_sb[:], emv_d[:])
                ident = ph1w.tile([128, 128], F32)
                make_identity(nc, ident[:])
                vT = ph1w.tile([64, S], F32)

                with tc.tile_pool(name="ps1", bufs=2, space="PSUM") as ps1:
                    for b in range(NB):
                        xt_t = xtp.tile([128, HC, 512], F32R, tag="xt")
                        nc.sync.dma_start(xt_t[:], xt[b])
                        pq0 = ps1.tile([128, 512], F32, tag="q0")
                        pq1 = ps1.tile([128, 512], F32, tag="q1")
                        pkv = ps1.tile([128, 512], F32, tag="kv")
                        for c in range(HC):
                            st, sp = c == 0, c == HC - 1
                            nc.tensor.matmul(
                                pq0[:], wq_sb[:, c, 0:128], xt_t[:, c, :], start=st, stop=sp
                            )
                            nc.tensor.matmul(
                                pq1[:], wq_sb[:, c, 128:256], xt_t[:, c, :], start=st, stop=sp
                            )
                            nc.tensor.matmul(
                                pkv[:], wkv_sb[:, c, :], xt_t[:, c, :], start=st, stop=sp
                            )
                        sb = ts(b, 512)
                        for p, pq in ((0, pq0), (1, pq1)):
                            rot = ph1t.tile([128, 512], F32, tag="rot")
                            nc.vector.tensor_copy(rot[0:32, :], pq[32:64, :])
                            nc.vector.tensor_copy(rot[32:64, :], pq[0:32, :])
                            nc.vector.tensor_copy(rot[64:96, :], pq[96:128, :])
                            nc.vector.tensor_copy(rot[96:128, :], pq[64:96, :])
                            qc = ph1t.tile([128, 512], F32, tag="qc")
                            nc.vector.tensor_tensor(qc[:], pq[:], cos_sb[:, sb], MUL)
                            qs = ph1t.tile([128, 512], F32, tag="qs")
                            nc.vector.tensor_tensor(qs[:], rot[:], sin_sb[:, sb], MUL)
                            nc.vector.tensor_tensor(qT[:, p, sb], qc[:], qs[:], ADD)
                        rotk = ph1t.tile([64, 512], F32, tag="rotk")
                        nc.vector.tensor_copy(rotk[0:32, :], pkv[32:64, :])
                        nc.vector.tensor_copy(rotk[32:64, :], pkv[0:32, :])
                        kc = ph1t.tile([64, 512], F32, tag="kc")
                        nc.vector.tensor_tensor(kc[:], pkv[0:64, :], cos_sb[0:64, sb], MUL)
                        ks = ph1t.tile([64, 512], F32, tag="ks")
                        nc.vector.tensor_tensor(ks[:], rotk[:], sin_sb[0:64, sb], MUL)
                        nc.vector.tensor_tensor(kTd[0:64, sb], kc[:], ks[:], ADD)
                        nc.vector.tensor_tensor(kTd[64:128, sb], kc[:], ks[:], ADD)
                        nc.vector.tensor_copy(vT[:, sb], pkv[64:128, :])

                with tc.tile_pool(name="psvt", bufs=2, space="PSUM") as psvt:
                    for i in range(HC):
                        pt = psvt.tile([128, 64], F32, tag="vt")
                        nc.tensor.transpose(pt[:], vT[:, ts(i, 128)], ident[0:64, 0:64])
                        nc.vector.tensor_scalar_mul(
                            V[:, i, 0:64], pt[:], emv_sb[:, i : i + 1]
                        )

            # ---- Phases 2+3 share PSUM so they can overlap ----
            with (
                tc.tile_pool(name="expp", bufs=2) as expp,
                tc.tile_pool(name="ph2t", bufs=3) as ph2t,
                tc.tile_pool(name="csd", bufs=4, space="DRAM") as csd,
                tc.tile_pool(name="ph3w", bufs=1) as ph3w,
                tc.tile_pool(name="osp", bufs=4) as osp,
                tc.tile_pool(name="pssc", bufs=2, space="PSUM") as pssc,
                tc.tile_pool(name="psav", bufs=1, space="PSUM") as psav,
                tc.tile_pool(name="pso", bufs=2, space="PSUM") as pso_,
            ):
                wo_sb = ph3w.tile([128, 2, H], F32R)
                nc.sync.dma_start(wo_sb[:], wo[:])
                for p in range(2):
                    for j in range(NB):
                        jb = ts(j, 512)
                        eA = expp.tile([128, HC, 512], F32R, tag="eA")
                        eB = expp.tile([128, HC, 512], F32R, tag="eB")
                        for i in range(HC):
                            psA = pssc.tile([128, 512], F32, tag="sA")
                            psB = pssc.tile([128, 512], F32, tag="sB")
                            nc.tensor.matmul(
                                psA[:], kTd[0:64, ts(i, 128)], qT[0:64, p, jb],
                                start=True, stop=True,
                            )
                            nc.tensor.matmul(
                                psB[:], kTd[64:128, ts(i, 128)], qT[64:128, p, jb],
                                start=True, stop=True,
                            )
                            nc.scalar.activation(eA[:, i, :], psA[:], EXP, scale=s_qk)
                            nc.scalar.activation(eB[:, i, :], psB[:], EXP, scale=s_qk)
                        pA = psav.tile([65, 512], F32, tag="avA")
                        pB = psav.tile([65, 512], F32, tag="avB")
                        for i in range(HC):
                            st, sp = i == 0, i == HC - 1
                            nc.tensor.matmul(pA[:], V[:, i, :], eA[:, i, :], start=st, stop=sp)
                            nc.tensor.matmul(pB[:], V[:, i, :], eB[:, i, :], start=st, stop=sp)
                        for h, pav in ((0, pA), (1, pB)):
                            cs = ph2t.tile([1, 512], F32, tag="cs")
                            nc.vector.tensor_copy(cs[:], pav[64:65, :])
                            cs_dram = csd.tile([1, 512], F32, tag="csd")
                            nc.sync.dma_start(cs_dram[:], cs[:])
                            cb = ph2t.tile([64, 512], F32, tag="cb")
                            nc.sync.dma_start(cb[:], cs_dram[:].to_broadcast((64, 512)))
                            rc = ph2t.tile([64, 512], F32, tag="rc")
                            nc.vector.reciprocal(rc[:], cb[:])
                            nc.vector.tensor_tensor(
                                aoT[h * 64 : (h + 1) * 64, p, jb], pav[0:64, :], rc[:], MUL
                            )

                # ---- Phase 3: output projection (row-parallel partial) ----
                for jq in range(16):
                    for hb in range(4):
                        po = pso_.tile([128, 512], F32, tag="po")
                        nc.tensor.matmul(
                            po[:], aoT[:, 0, ts(jq, 128)], wo_sb[:, 0, ts(hb, 512)],
                            start=True, stop=False,
                        )
                        nc.tensor.matmul(
                            po[:], aoT[:, 1, ts(jq, 128)], wo_sb[:, 1, ts(hb, 512)],
                            start=False, stop=True,
                        )
                        ob = osp.tile([128, 512], F32, tag="ob")
                        if (jq * 4 + hb) % 2 == 0:
                            nc.vector.tensor_copy(ob[:], po[:])
                        else:
                            nc.scalar.copy(ob[:], po[:])
                        nc.sync.dma_start(outp[ts(jq, 128), ts(hb, 512)], ob[:])

    nc.compile()
    return nc


def kernel(
    hidden_states,
    attention_mask,
    position_ids,
    wq,
    wk,
    wv,
    wo,
    _trace=False,
):
    global LAST_EXEC_NS, LAST_TRACE
    x = np.asarray(hidden_states, np.float32)[0]  # [S, H]
    mask = np.asarray(attention_mask, np.float32)[0]  # [S]
    pos = np.asarray(position_ids)[0].astype(np.float32)  # [S]

    wq_t, s_q = _ternarize(wq)
    wk_t, s_k = _ternarize(wk)
    wv_t, s_v = _ternarize(wv)
    wo_t, s_o = _ternarize(wo)
    s_qk = float(np.float32(s_q) * np.float32(s_k) / np.float32(8.0))
    s_vo = np.float32(s_v) * np.float32(s_o)

    key = ("v2", s_qk)
    if key not in _CACHE:
        _CACHE.clear()
        _CACHE[key] = _build_program(s_qk)
    nc = _CACHE[key]

    # shared inputs
    xt_host = np.ascontiguousarray(
        x.T.reshape(HC, 128, NB, 512).transpose(2, 1, 0, 3)
    )
    inv = (
        1.0 / (10000.0 ** (np.arange(0, D, 2, dtype=np.float32) / np.float32(D)))
    ).astype(np.float32)
    fr = pos[:, None] * inv[None, :]  # [S, 32]
    emb = np.concatenate([fr, fr], axis=1)  # [S, 64]
    cos64 = np.cos(emb).astype(np.float32)
    sin64 = np.sin(emb).astype(np.float32)
    sin64[:, : D // 2] *= -1.0
    cos128 = np.ascontiguousarray(np.vstack([cos64.T, cos64.T]))  # [128, S]
    sin128 = np.ascontiguousarray(np.vstack([sin64.T, sin64.T]))
    expmask = np.exp(mask).astype(np.float32)  # [S]
    em_r = np.ascontiguousarray(expmask.reshape(HC, 128).T)  # [128, HC]
    emv_r = np.ascontiguousarray((expmask * s_vo).reshape(HC, 128).T)

    in_maps = []
    for c in range(NCORES):
        wq_c = np.ascontiguousarray(
            wq_t[c * OC : (c + 1) * OC, :].T.reshape(HC, 128, OC).transpose(1, 0, 2)
        )
        wk_c = wk_t[c * D : (c + 1) * D, :].T  # [H, 64]
        wv_c = wv_t[c * D : (c + 1) * D, :].T
        wkv_c = np.ascontiguousarray(
            np.concatenate([wk_c, wv_c], axis=1).reshape(HC, 128, 128).transpose(1, 0, 2)
        )
        wo_c = np.ascontiguousarray(
            wo_t[:, c * OC : (c + 1) * OC].T.reshape(2, 128, H).transpose(1, 0, 2)
        )
        in_maps.append(
            {
                "xt": xt_host,
                "wq_t": wq_c,
                "wkv_t": wkv_c,
                "wo_t": wo_c,
                "cos_t": cos128,
                "sin_t": sin128,
                "emv_t": emv_r,
                "em_t": em_r,
            }
        )

    res = run_bass_kernel_spmd(
        nc, in_maps, core_ids=list(range(NCORES)), trace=bool(_trace)
    )
    LAST_EXEC_NS = res.exec_time_ns
    LAST_TRACE = res.instructions_and_trace[1] if res.instructions_and_trace else None

    out = res.results[0]["outp"].astype(np.float32)
    for c in range(1, NCORES):
        out = out + res.results[c]["outp"]
    return out.reshape(1, S, H).astype(np.float32)


# revision 4
# speedup vs baseline: 1.3492x; 1.0398x over previous
"""BitNet attention (GQA + RoPE) on 8 Trainium2 NeuronCores.

Tensor-parallel over heads: core c owns q-heads [4c, 4c+4), kv-head c.
Each core computes q/k/v projections (ternary BitNet weights), RoPE,
attention for its heads, and a row-parallel partial of the Wo
projection; the host sums the 8 partials.

Dtype strategy (measured on HW):
  - projections / AV / Wo matmuls: float32r (~fp32 precision at
    1 cyc/row for moving dims >= 256)
  - scores matmuls: bf16, K=64 row-tiled so the two heads of a pair
    run concurrently in the PE array (55 ns per [64x128x512] matmul)
  - fp32r with row tiling is ~20x slow on HW; bf16 scores cost ~9e-4
    relative error end-to-end.

The attention mask is folded into the V tile: attn = exp(s*qk + m) =
exp(m)*exp(s*qk), so V rows and the denominator-ones column are
pre-scaled by exp(mask) and the EXP activation needs no bias.

Layout notes (per core):
  qT   [128, 2, 2048]  head-pair p: head 2p on partitions 0:64, head
                       2p+1 on 64:128; RoPE applied; bf16.
  kTd  [128, 2048]     kv head duplicated on both partition halves
                       (lhsT of both row-tiled score matmuls); bf16.
  V    [128, 16, 65]   [sk-chunk, 65] f32r; col 64 = exp(mask) so the
                       AV matmul also emits softmax denominators;
                       cols 0:64 scaled by exp(mask)*s_v*s_o.
  aoT  [128, 2, 2048]  normalized attention outputs, o-major, f32r,
                       lhsT of the Wo matmul.
"""

import sys

if "/opt/trn_rl_repo" not in sys.path:
    sys.path.insert(0, "/opt/trn_rl_repo")

import numpy as np

import concourse.bass as bass
from concourse import bacc, mybir
from concourse.bass import ts
from concourse.bass_utils import run_bass_kernel_spmd
from concourse.masks import make_identity
from concourse.tile import TileContext

F32 = mybir.dt.float32
F32R = mybir.dt.float32r
BF16 = mybir.dt.bfloat16

S = 2048
H = 2048
N_HEADS = 32
N_KV = 8
D = 64
NCORES = 8
HPC = N_HEADS // NCORES  # 4 q heads per core
OC = HPC * D  # 256 output dims per core
NB = S // 512  # 4 s-blocks of 512
HC = H // 128  # 16 hidden chunks

LAST_EXEC_NS = None
LAST_TRACE = None
_CACHE = {}


def _ternarize(w):
    w = np.asarray(w, np.float32)
    s = (np.abs(w).mean() + np.float32(1e-6)).astype(np.float32)
    t = np.round(np.clip(w / s, np.float32(-1.0), np.float32(1.0))).astype(np.float32)
    return t, float(s)


def _build_program(s_qk):
    nc = bacc.Bacc("TRN2", target_bir_lowering=False, debug=False, num_devices=NCORES)

    xt = nc.dram_tensor("xt", [NB, 128, HC, 512], F32R, kind="ExternalInput")
    wq = nc.dram_tensor("wq_t", [128, HC, OC], F32R, kind="ExternalInput")
    wkv = nc.dram_tensor("wkv_t", [128, HC, 128], F32R, kind="ExternalInput")
    wo = nc.dram_tensor("wo_t", [128, 2, H], F32R, kind="ExternalInput")
    cos_d = nc.dram_tensor("cos_t", [128, S], F32, kind="ExternalInput")
    sin_d = nc.dram_tensor("sin_t", [128, S], F32, kind="ExternalInput")
    emv_d = nc.dram_tensor("emv_t", [128, HC], F32, kind="ExternalInput")
    em_d = nc.dram_tensor("em_t", [128, HC], F32R, kind="ExternalInput")
    outp = nc.dram_tensor("outp", [S, H], F32, kind="ExternalOutput")

    EXP = mybir.ActivationFunctionType.Exp
    MUL = mybir.AluOpType.mult
    ADD = mybir.AluOpType.add

    with TileContext(nc) as tc:
        with tc.tile_pool(name="persist", bufs=1) as persist:
            qT = persist.tile([128, 2, S], BF16)
            kTd = persist.tile([128, S], BF16)
            V = persist.tile([128, HC, 65], F32R)
            aoT = persist.tile([128, 2, S], F32R)
            for i in range(HC):
                nc.sync.dma_start(V[:, i, 64:65], em_d[:, i : i + 1])

            # ---- Phase 1: projections + RoPE ----
            with (
                tc.tile_pool(name="ph1w", bufs=1) as ph1w,
                tc.tile_pool(name="xtp", bufs=2) as xtp,
                tc.tile_pool(name="ph1t", bufs=3) as ph1t,
            ):
                wq_sb = ph1w.tile([128, HC, OC], F32R)
                wkv_sb = ph1w.tile([128, HC, 128], F32R)
                for c in range(HC):
                    nc.sync.dma_start(wq_sb[:, c, :], wq[:, c, :])
                    nc.sync.dma_start(wkv_sb[:, c, :], wkv[:, c, :])
                cos_sb = ph1w.tile([128, S], F32)
                sin_sb = ph1w.tile([128, S], F32)
                for c in range(4):
                    nc.sync.dma_start(cos_sb[:, ts(c, 512)], cos_d[:, ts(c, 512)])
                    nc.sync.dma_start(sin_sb[:, ts(c, 512)], sin_d[:, ts(c, 512)])
                emv_sb = ph1w.tile([128, HC], F32)
                nc.sync.dma_start(emv_sb[:], emv_d[:])
                ident = ph1w.tile([128, 128], F32)
                make_identity(nc, ident[:])
                vT = ph1w.tile([64, S], F32)

                with tc.tile_pool(name="ps1", bufs=2, space="PSUM") as ps1:
                    for b in range(NB):
                        xt_t = xtp.tile([128, HC, 512], F32R, tag="xt")
                        for c4 in range(4):
                            nc.sync.dma_start(
                                xt_t[:, ts(c4, 4), :], xt[b, :, ts(c4, 4), :]
                            )
                        pq0 = ps1.tile([128, 512], F32, tag="q0")
                        pq1 = ps1.tile([128, 512], F32, tag="q1")
                        pkv = ps1.tile([128, 512], F32, tag="kv")
                        for c in range(HC):
                            st, sp = c == 0, c == HC - 1
                            nc.tensor.matmul(
                                pq0[:], wq_sb[:, c, 0:128], xt_t[:, c, :], start=st, stop=sp
                            )
                            nc.tensor.matmul(
                                pq1[:], wq_sb[:, c, 128:256], xt_t[:, c, :], start=st, stop=sp
                            )
                            nc.tensor.matmul(
                                pkv[:], wkv_sb[:, c, :], xt_t[:, c, :], start=st, stop=sp
                            )
                        sb = ts(b, 512)
                        for p, pq in ((0, pq0), (1, pq1)):
                            rot = ph1t.tile([128, 512], F32, tag="rot")
                            nc.vector.tensor_copy(rot[0:32, :], pq[32:64, :])
                            nc.vector.tensor_copy(rot[32:64, :], pq[0:32, :])
                            nc.vector.tensor_copy(rot[64:96, :], pq[96:128, :])
                            nc.vector.tensor_copy(rot[96:128, :], pq[64:96, :])
                            qc = ph1t.tile([128, 512], F32, tag="qc")
                            nc.vector.tensor_tensor(qc[:], pq[:], cos_sb[:, sb], MUL)
                            qs = ph1t.tile([128, 512], F32, tag="qs")
                            nc.vector.tensor_tensor(qs[:], rot[:], sin_sb[:, sb], MUL)
                            nc.vector.tensor_tensor(qT[:, p, sb], qc[:], qs[:], ADD)
                        rotk = ph1t.tile([64, 512], F32, tag="rotk")
                        nc.vector.tensor_copy(rotk[0:32, :], pkv[32:64, :])
                        nc.vector.tensor_copy(rotk[32:64, :], pkv[0:32, :])
                        kc = ph1t.tile([64, 512], F32, tag="kc")
                        nc.vector.tensor_tensor(kc[:], pkv[0:64, :], cos_sb[0:64, sb], MUL)
                        ks = ph1t.tile([64, 512], F32, tag="ks")
                        nc.vector.tensor_tensor(ks[:], rotk[:], sin_sb[0:64, sb], MUL)
                        nc.vector.tensor_tensor(kTd[0:64, sb], kc[:], ks[:], ADD)
                        nc.vector.tensor_tensor(kTd[64:128, sb], kc[:], ks[:], ADD)
                        nc.vector.tensor_copy(vT[:, sb], pkv[64:128, :])

                with tc.tile_pool(name="psvt", bufs=2, space="PSUM") as psvt:
                    for i in range(HC):
                        pt = psvt.tile([128, 64], F32, tag="vt")
                        nc.tensor.transpose(pt[:], vT[:, ts(i, 128)], ident[0:64, 0:64])
                        nc.vector.tensor_scalar_mul(
                            V[:, i, 0:64], pt[:], emv_sb[:, i : i + 1]
                        )

            # ---- Phases 2+3 share PSUM so they can overlap ----
            with (
                tc.tile_pool(name="expp", bufs=2) as expp,
                tc.tile_pool(name="ph2t", bufs=3) as ph2t,
                tc.tile_pool(name="csd", bufs=4, space="DRAM") as csd,
                tc.tile_pool(name="ph3w", bufs=1) as ph3w,
                tc.tile_pool(name="osp", bufs=4) as osp,
                tc.tile_pool(name="pssc", bufs=1, space="PSUM") as pssc,
                tc.tile_pool(name="psav", bufs=2, space="PSUM") as psav,
                tc.tile_pool(name="pso", bufs=2, space="PSUM") as pso_,
            ):
                wo_sb = ph3w.tile([128, 2, H], F32R)
                for k2 in range(2):
                    for hb in range(4):
                        nc.sync.dma_start(
                            wo_sb[:, k2, ts(hb, 512)], wo[:, k2, ts(hb, 512)]
                        )
                for p in range(2):
                    for j in range(NB):
                        jb = ts(j, 512)
                        eA = expp.tile([128, HC, 512], F32R, tag="eA")
                        eB = expp.tile([128, HC, 512], F32R, tag="eB")
                        for i in range(HC):
                            psA = pssc.tile([128, 512], F32, tag="sA")
                            psB = pssc.tile([128, 512], F32, tag="sB")
                            nc.tensor.matmul(
                                psA[:], kTd[0:64, ts(i, 128)], qT[0:64, p, jb],
                                start=True, stop=True,
                            )
                            nc.tensor.matmul(
                                psB[:], kTd[64:128, ts(i, 128)], qT[64:128, p, jb],
                                start=True, stop=True,
                            )
                            nc.scalar.activation(eA[:, i, :], psA[:], EXP, scale=s_qk)
                            nc.scalar.activation(eB[:, i, :], psB[:], EXP, scale=s_qk)
                        pA = psav.tile([65, 512], F32, tag="avA")
                        pB = psav.tile([65, 512], F32, tag="avB")
                        for i in range(HC):
                            st, sp = i == 0, i == HC - 1
                            nc.tensor.matmul(pA[:], V[:, i, :], eA[:, i, :], start=st, stop=sp)
                            nc.tensor.matmul(pB[:], V[:, i, :], eB[:, i, :], start=st, stop=sp)
                        cs = ph2t.tile([33, 512], F32, tag="cs")
                        nc.vector.tensor_copy(cs[0:1, :], pA[64:65, :])
                        nc.vector.tensor_copy(cs[32:33, :], pB[64:65, :])
                        rcs = ph2t.tile([33, 512], F32, tag="rcs")
                        nc.vector.reciprocal(rcs[:], cs[:])
                        cs_dram = csd.tile([2, 1, 512], F32, tag="csd")
                        nc.sync.dma_start(cs_dram[0], rcs[0:1, :])
                        nc.sync.dma_start(cs_dram[1], rcs[32:33, :])
                        for h, pav in ((0, pA), (1, pB)):
                            cb = ph2t.tile([64, 512], F32, tag="cb")
                            nc.sync.dma_start(
                                cb[:], cs_dram[h].to_broadcast((64, 512))
                            )
                            nc.vector.tensor_tensor(
                                aoT[h * 64 : (h + 1) * 64, p, jb], pav[0:64, :], cb[:], MUL
                            )

                # ---- Phase 3: output projection (row-parallel partial) ----
                for jq in range(16):
                    for hb in range(4):
                        po = pso_.tile([128, 512], F32, tag="po")
                        nc.tensor.matmul(
                            po[:], aoT[:, 0, ts(jq, 128)], wo_sb[:, 0, ts(hb, 512)],
                            start=True, stop=False,
                        )
                        nc.tensor.matmul(
                            po[:], aoT[:, 1, ts(jq, 128)], wo_sb[:, 1, ts(hb, 512)],
                            start=False, stop=True,
                        )
                        ob = osp.tile([128, 512], F32, tag="ob")
                        nc.vector.tensor_copy(ob[:], po[:])
                        nc.sync.dma_start(outp[ts(jq, 128), ts(hb, 512)], ob[:])

    nc.compile()
    return nc


def kernel(
    hidden_states,
    attention_mask,
    position_ids,
    wq,
    wk,
    wv,
    wo,
    _trace=False,
):
    global LAST_EXEC_NS, LAST_TRACE
    x = np.asarray(hidden_states, np.float32)[0]  # [S, H]
    mask = np.asarray(attention_mask, np.float32)[0]  # [S]
    pos = np.asarray(position_ids)[0].astype(np.float32)  # [S]

    wq_t, s_q = _ternarize(wq)
    wk_t, s_k = _ternarize(wk)
    wv_t, s_v = _ternarize(wv)
    wo_t, s_o = _ternarize(wo)
    s_qk = float(np.float32(s_q) * np.float32(s_k) / np.float32(8.0))
    s_vo = np.float32(s_v) * np.float32(s_o)

    key = ("v3", s_qk)
    if key not in _CACHE:
        _CACHE.clear()
        _CACHE[key] = _build_program(s_qk)
    nc = _CACHE[key]

    # shared inputs
    xt_host = np.ascontiguousarray(
        x.T.reshape(HC, 128, NB, 512).transpose(2, 1, 0, 3)
    )
    inv = (
        1.0 / (10000.0 ** (np.arange(0, D, 2, dtype=np.float32) / np.float32(D)))
    ).astype(np.float32)
    fr = pos[:, None] * inv[None, :]  # [S, 32]
    emb = np.concatenate([fr, fr], axis=1)  # [S, 64]
    cos64 = np.cos(emb).astype(np.float32)
    sin64 = np.sin(emb).astype(np.float32)
    sin64[:, : D // 2] *= -1.0
    cos128 = np.ascontiguousarray(np.vstack([cos64.T, cos64.T]))  # [128, S]
    sin128 = np.ascontiguousarray(np.vstack([sin64.T, sin64.T]))
    expmask = np.exp(mask).astype(np.float32)  # [S]
    em_r = np.ascontiguousarray(expmask.reshape(HC, 128).T)  # [128, HC]
    emv_r = np.ascontiguousarray((expmask * s_vo).reshape(HC, 128).T)

    in_maps = []
    for c in range(NCORES):
        wq_c = np.ascontiguousarray(
            wq_t[c * OC : (c + 1) * OC, :].T.reshape(HC, 128, OC).transpose(1, 0, 2)
        )
        wk_c = wk_t[c * D : (c + 1) * D, :].T  # [H, 64]
        wv_c = wv_t[c * D : (c + 1) * D, :].T
        wkv_c = np.ascontiguousarray(
            np.concatenate([wk_c, wv_c], axis=1).reshape(HC, 128, 128).transpose(1, 0, 2)
        )
        wo_c = np.ascontiguousarray(
            wo_t[:, c * OC : (c + 1) * OC].T.reshape(2, 128, H).transpose(1, 0, 2)
        )
        in_maps.append(
            {
                "xt": xt_host,
                "wq_t": wq_c,
                "wkv_t": wkv_c,
                "wo_t": wo_c,
                "cos_t": cos128,
                "sin_t": sin128,
                "emv_t": emv_r,
                "em_t": em_r,
            }
        )

    res = run_bass_kernel_spmd(
        nc, in_maps, core_ids=list(range(NCORES)), trace=bool(_trace)
    )
    LAST_EXEC_NS = res.exec_time_ns
    LAST_TRACE = res.instructions_and_trace[1] if res.instructions_and_trace else None

    out = res.results[0]["outp"].astype(np.float32)
    for c in range(1, NCORES):
        out = out + res.results[c]["outp"]
    return out.reshape(1, S, H).astype(np.float32)


# revision 5
# speedup vs baseline: 1.7416x; 1.2909x over previous
"""BitNet attention (GQA + RoPE) on 8 Trainium2 NeuronCores.

Tensor-parallel over heads: core c owns q-heads [4c, 4c+4), kv-head c.
Each core computes q/k/v projections (ternary BitNet weights), RoPE,
attention for its heads, and a row-parallel partial of the Wo
projection; the host sums the 8 partials.

Dtype strategy (measured on HW):
  - projections / AV / Wo matmuls: float32r (~fp32 precision at
    1 cyc/row for moving dims >= 256)
  - scores matmuls: bf16, K=64 row-tiled so the two heads of a pair
    run concurrently in the PE array (55 ns per [64x128x512] matmul)
  - fp32r with row tiling is ~20x slow on HW; bf16 scores cost ~9e-4
    relative error end-to-end.

The attention mask is folded into the V tile: attn = exp(s*qk + m) =
exp(m)*exp(s*qk), so V rows and the denominator-ones column are
pre-scaled by exp(mask) and the EXP activation needs no bias.

Layout notes (per core):
  qT   [128, 2, 2048]  head-pair p: head 2p on partitions 0:64, head
                       2p+1 on 64:128; RoPE applied; bf16.
  kTd  [128, 2048]     kv head duplicated on both partition halves
                       (lhsT of both row-tiled score matmuls); bf16.
  V    [128, 16, 65]   [sk-chunk, 65] f32r; col 64 = exp(mask) so the
                       AV matmul also emits softmax denominators;
                       cols 0:64 scaled by exp(mask)*s_v*s_o.
  aoT  [128, 2, 2048]  normalized attention outputs, o-major, f32r,
                       lhsT of the Wo matmul.
"""

import sys

if "/opt/trn_rl_repo" not in sys.path:
    sys.path.insert(0, "/opt/trn_rl_repo")

import numpy as np

import concourse.bass as bass
from concourse import bacc, mybir
from concourse.bass import ts
from concourse.bass_utils import run_bass_kernel_spmd
from concourse.masks import make_identity
from concourse.tile import TileContext

F32 = mybir.dt.float32
F32R = mybir.dt.float32r
BF16 = mybir.dt.bfloat16

S = 2048
H = 2048
N_HEADS = 32
N_KV = 8
D = 64
NCORES = 8
HPC = N_HEADS // NCORES  # 4 q heads per core
OC = HPC * D  # 256 output dims per core
NB = S // 512  # 4 s-blocks of 512
HC = H // 128  # 16 hidden chunks

LAST_EXEC_NS = None
LAST_TRACE = None
_CACHE = {}


def _ternarize(w):
    w = np.asarray(w, np.float32)
    s = (np.abs(w).mean() + np.float32(1e-6)).astype(np.float32)
    t = np.round(np.clip(w / s, np.float32(-1.0), np.float32(1.0))).astype(np.float32)
    return t, float(s)


def _build_program(s_qk):
    nc = bacc.Bacc("TRN2", target_bir_lowering=False, debug=False, num_devices=NCORES)

    xt = nc.dram_tensor("xt", [NB, 128, HC, 512], F32R, kind="ExternalInput")
    wq = nc.dram_tensor("wq_t", [128, HC, OC], F32R, kind="ExternalInput")
    wkv = nc.dram_tensor("wkv_t", [128, HC, 128], F32R, kind="ExternalInput")
    wo = nc.dram_tensor("wo_t", [128, 2, H], F32R, kind="ExternalInput")
    cos_d = nc.dram_tensor("cos_t", [128, S], F32, kind="ExternalInput")
    sin_d = nc.dram_tensor("sin_t", [128, S], F32, kind="ExternalInput")
    emv_d = nc.dram_tensor("emv_t", [128, HC], F32, kind="ExternalInput")
    em_d = nc.dram_tensor("em_t", [128, HC], F32R, kind="ExternalInput")
    outp = nc.dram_tensor("outp", [S, H], F32, kind="ExternalOutput")

    EXP = mybir.ActivationFunctionType.Exp
    MUL = mybir.AluOpType.mult
    ADD = mybir.AluOpType.add

    with TileContext(nc) as tc:
        with tc.tile_pool(name="persist", bufs=1) as persist:
            qT = persist.tile([128, 2, S], BF16)
            kTd = persist.tile([128, S], BF16)
            V = persist.tile([128, HC, 65], F32R)
            aoT = persist.tile([128, 2, S], F32R)
            for i in range(HC):
                nc.sync.dma_start(V[:, i, 64:65], em_d[:, i : i + 1])

            # ---- Phase 1: projections + RoPE ----
            with (
                tc.tile_pool(name="ph1w", bufs=1) as ph1w,
                tc.tile_pool(name="xtp", bufs=2) as xtp,
                tc.tile_pool(name="ph1t", bufs=3) as ph1t,
            ):
                wq_sb = ph1w.tile([128, HC, OC], F32R)
                wkv_sb = ph1w.tile([128, HC, 128], F32R)
                for c in range(HC):
                    nc.sync.dma_start(wq_sb[:, c, :], wq[:, c, :])
                    nc.sync.dma_start(wkv_sb[:, c, :], wkv[:, c, :])
                cos_sb = ph1w.tile([128, S], F32)
                sin_sb = ph1w.tile([128, S], F32)
                for c in range(4):
                    nc.sync.dma_start(cos_sb[:, ts(c, 512)], cos_d[:, ts(c, 512)])
                    nc.sync.dma_start(sin_sb[:, ts(c, 512)], sin_d[:, ts(c, 512)])
                emv_sb = ph1w.tile([128, HC], F32)
                nc.sync.dma_start(emv_sb[:], emv_d[:])
                ident = ph1w.tile([128, 128], F32)
                make_identity(nc, ident[:])
                vT = ph1w.tile([64, S], F32)

                with (
                    tc.tile_pool(name="ps1", bufs=2, space="PSUM") as ps1,
                    tc.tile_pool(name="psvt", bufs=2, space="PSUM") as psvt,
                ):
                    for b in range(NB):
                        xt_t = xtp.tile([128, HC, 512], F32R, tag="xt")
                        for c4 in range(4):
                            nc.sync.dma_start(
                                xt_t[:, ts(c4, 4), :], xt[b, :, ts(c4, 4), :]
                            )
                        pq0 = ps1.tile([128, 512], F32, tag="q0")
                        pq1 = ps1.tile([128, 512], F32, tag="q1")
                        pkv = ps1.tile([128, 512], F32, tag="kv")
                        for c in range(HC):
                            st, sp = c == 0, c == HC - 1
                            nc.tensor.matmul(
                                pq0[:], wq_sb[:, c, 0:128], xt_t[:, c, :], start=st, stop=sp
                            )
                            nc.tensor.matmul(
                                pq1[:], wq_sb[:, c, 128:256], xt_t[:, c, :], start=st, stop=sp
                            )
                            nc.tensor.matmul(
                                pkv[:], wkv_sb[:, c, :], xt_t[:, c, :], start=st, stop=sp
                            )
                        sb = ts(b, 512)
                        for p, pq in ((0, pq0), (1, pq1)):
                            rot = ph1t.tile([128, 512], F32, tag="rot")
                            nc.vector.tensor_copy(rot[0:32, :], pq[32:64, :])
                            nc.vector.tensor_copy(rot[32:64, :], pq[0:32, :])
                            nc.vector.tensor_copy(rot[64:96, :], pq[96:128, :])
                            nc.vector.tensor_copy(rot[96:128, :], pq[64:96, :])
                            qc = ph1t.tile([128, 512], F32, tag="qc")
                            nc.vector.tensor_tensor(qc[:], pq[:], cos_sb[:, sb], MUL)
                            qs = ph1t.tile([128, 512], F32, tag="qs")
                            nc.vector.tensor_tensor(qs[:], rot[:], sin_sb[:, sb], MUL)
                            nc.vector.tensor_tensor(qT[:, p, sb], qc[:], qs[:], ADD)
                        rotk = ph1t.tile([64, 512], F32, tag="rotk")
                        nc.vector.tensor_copy(rotk[0:32, :], pkv[32:64, :])
                        nc.vector.tensor_copy(rotk[32:64, :], pkv[0:32, :])
                        kc = ph1t.tile([64, 512], F32, tag="kc")
                        nc.vector.tensor_tensor(kc[:], pkv[0:64, :], cos_sb[0:64, sb], MUL)
                        ks = ph1t.tile([64, 512], F32, tag="ks")
                        nc.vector.tensor_tensor(ks[:], rotk[:], sin_sb[0:64, sb], MUL)
                        nc.vector.tensor_tensor(kTd[0:64, sb], kc[:], ks[:], ADD)
                        nc.vector.tensor_tensor(kTd[64:128, sb], kc[:], ks[:], ADD)
                        nc.vector.tensor_copy(vT[:, sb], pkv[64:128, :])
                        for i4 in range(4):
                            i = 4 * b + i4
                            pt = psvt.tile([128, 64], F32, tag="vt")
                            nc.tensor.transpose(
                                pt[:], vT[:, ts(i, 128)], ident[0:64, 0:64]
                            )
                            nc.vector.tensor_scalar_mul(
                                V[:, i, 0:64], pt[:], emv_sb[:, i : i + 1]
                            )

            # ---- Phases 2+3, interleaved: j-blocks outer, Wo per block ----
            with (
                tc.tile_pool(name="expp", bufs=1) as expp,
                tc.tile_pool(name="ph2t", bufs=3) as ph2t,
                tc.tile_pool(name="csd", bufs=4, space="DRAM") as csd,
                tc.tile_pool(name="ph3w", bufs=1) as ph3w,
                tc.tile_pool(name="osp", bufs=4) as osp,
                tc.tile_pool(name="pssc", bufs=2, space="PSUM") as pssc,
                tc.tile_pool(name="psav", bufs=1, space="PSUM") as psav,
                tc.tile_pool(name="pso", bufs=2, space="PSUM") as pso_,
            ):
                wo_sb = ph3w.tile([128, 2, H], F32R)
                for k2 in range(2):
                    for hb in range(4):
                        nc.sync.dma_start(
                            wo_sb[:, k2, ts(hb, 512)], wo[:, k2, ts(hb, 512)]
                        )
                for j in range(NB):
                    jb = ts(j, 512)
                    for p in range(2):
                        e2 = expp.tile([128, HC, 1024], F32R, tag="e2")
                        pA = psav.tile([65, 512], F32, tag="avA")
                        pB = psav.tile([65, 512], F32, tag="avB")
                        for i in range(HC):
                            psAB = pssc.tile([128, 1024], F32, tag="sAB")
                            nc.tensor.matmul(
                                psAB[:, 0:512], kTd[0:64, ts(i, 128)], qT[0:64, p, jb],
                                start=True, stop=True,
                            )
                            nc.tensor.matmul(
                                psAB[:, 512:1024], kTd[64:128, ts(i, 128)],
                                qT[64:128, p, jb], start=True, stop=True,
                            )
                            nc.scalar.activation(e2[:, i, :], psAB[:], EXP, scale=s_qk)
                            st, sp = i == 0, i == HC - 1
                            nc.tensor.matmul(
                                pA[:], V[:, i, :], e2[:, i, 0:512], start=st, stop=sp
                            )
                            nc.tensor.matmul(
                                pB[:], V[:, i, :], e2[:, i, 512:1024], start=st, stop=sp
                            )
                        cs = ph2t.tile([33, 512], F32, tag="cs")
                        nc.vector.tensor_copy(cs[0:1, :], pA[64:65, :])
                        nc.vector.tensor_copy(cs[32:33, :], pB[64:65, :])
                        rcs = ph2t.tile([33, 512], F32, tag="rcs")
                        nc.vector.reciprocal(rcs[:], cs[:])
                        cs_dram = csd.tile([2, 1, 512], F32, tag="csd")
                        nc.sync.dma_start(cs_dram[0], rcs[0:1, :])
                        nc.sync.dma_start(cs_dram[1], rcs[32:33, :])
                        for h, pav in ((0, pA), (1, pB)):
                            cb = ph2t.tile([64, 512], F32, tag="cb")
                            nc.sync.dma_start(
                                cb[:], cs_dram[h].to_broadcast((64, 512))
                            )
                            nc.vector.tensor_tensor(
                                aoT[h * 64 : (h + 1) * 64, p, jb], pav[0:64, :], cb[:], MUL
                            )
                    # Wo for the 4 sq-chunks of this j-block
                    for jq4 in range(4):
                        jq = 4 * j + jq4
                        for hb in range(4):
                            po = pso_.tile([128, 512], F32, tag="po")
                            nc.tensor.matmul(
                                po[:], aoT[:, 0, ts(jq, 128)], wo_sb[:, 0, ts(hb, 512)],
                                start=True, stop=False,
                            )
                            nc.tensor.matmul(
                                po[:], aoT[:, 1, ts(jq, 128)], wo_sb[:, 1, ts(hb, 512)],
                                start=False, stop=True,
                            )
                            ob = osp.tile([128, 512], F32, tag="ob")
                            nc.vector.tensor_copy(ob[:], po[:])
                            nc.sync.dma_start(outp[ts(jq, 128), ts(hb, 512)], ob[:])

    nc.compile()
    return nc


def kernel(
    hidden_states,
    attention_mask,
    position_ids,
    wq,
    wk,
    wv,
    wo,
    _trace=False,
):
    global LAST_EXEC_NS, LAST_TRACE
    x = np.asarray(hidden_states, np.float32)[0]  # [S, H]
    mask = np.asarray(attention_mask, np.float32)[0]  # [S]
    pos = np.asarray(position_ids)[0].astype(np.float32)  # [S]

    wq_t, s_q = _ternarize(wq)
    wk_t, s_k = _ternarize(wk)
    wv_t, s_v = _ternarize(wv)
    wo_t, s_o = _ternarize(wo)
    s_qk = float(np.float32(s_q) * np.float32(s_k) / np.float32(8.0))
    s_vo = np.float32(s_v) * np.float32(s_o)

    key = ("v4", s_qk)
    if key not in _CACHE:
        _CACHE.clear()
        _CACHE[key] = _build_program(s_qk)
    nc = _CACHE[key]

    # shared inputs
    xt_host = np.ascontiguousarray(
        x.T.reshape(HC, 128, NB, 512).transpose(2, 1, 0, 3)
    )
    inv = (
        1.0 / (10000.0 ** (np.arange(0, D, 2, dtype=np.float32) / np.float32(D)))
    ).astype(np.float32)
    fr = pos[:, None] * inv[None, :]  # [S, 32]
    emb = np.concatenate([fr, fr], axis=1)  # [S, 64]
    cos64 = np.cos(emb).astype(np.float32)
    sin64 = np.sin(emb).astype(np.float32)
    sin64[:, : D // 2] *= -1.0
    cos128 = np.ascontiguousarray(np.vstack([cos64.T, cos64.T]))  # [128, S]
    sin128 = np.ascontiguousarray(np.vstack([sin64.T, sin64.T]))
    expmask = np.exp(mask).astype(np.float32)  # [S]
    em_r = np.ascontiguousarray(expmask.reshape(HC, 128).T)  # [128, HC]
    emv_r = np.ascontiguousarray((expmask * s_vo).reshape(HC, 128).T)

    in_maps = []
    for c in range(NCORES):
        wq_c = np.ascontiguousarray(
            wq_t[c * OC : (c + 1) * OC, :].T.reshape(HC, 128, OC).transpose(1, 0, 2)
        )
        wk_c = wk_t[c * D : (c + 1) * D, :].T  # [H, 64]
        wv_c = wv_t[c * D : (c + 1) * D, :].T
        wkv_c = np.ascontiguousarray(
            np.concatenate([wk_c, wv_c], axis=1).reshape(HC, 128, 128).transpose(1, 0, 2)
        )
        wo_c = np.ascontiguousarray(
            wo_t[:, c * OC : (c + 1) * OC].T.reshape(2, 128, H).transpose(1, 0, 2)
        )
        in_maps.append(
            {
                "xt": xt_host,
                "wq_t": wq_c,
                "wkv_t": wkv_c,
                "wo_t": wo_c,
                "cos_t": cos128,
                "sin_t": sin128,
                "emv_t": emv_r,
                "em_t": em_r,
            }
        )

    res = run_bass_kernel_spmd(
        nc, in_maps, core_ids=list(range(NCORES)), trace=bool(_trace)
    )
    LAST_EXEC_NS = res.exec_time_ns
    LAST_TRACE = res.instructions_and_trace[1] if res.instructions_and_trace else None

    out = res.results[0]["outp"].astype(np.float32)
    for c in range(1, NCORES):
        out = out + res.results[c]["outp"]
    return out.reshape(1, S, H).astype(np.float32)
